# revision 10
# baseline (speedup 1.0000x reference)
"""Trainium2 Bass kernel for nn_MixtureOfExpertsModel (moe_routing).

Computes, for x [65536, 1024] and 10 experts with 15 outputs each:
    miu = x @ expert_w^T + expert_b      (per expert)
    xi  = x @ gate_w^T + gate_b          (per expert)
    out = sum_e softmax_e(xi) * miu      -> [65536, 15]

Strategy: pure data parallel over 8 NeuronCores (8192 rows each); at fp16
the kernel is PE-bound (2.52 GMAC/core -> 153.6k PE cycles = 64 us), so the
design keeps the PE streaming 300-column fp16 matmuls back-to-back at the
~127 ns issue floor and pushes everything else off the critical path:

 * x is repacked on the host into per-slab blocks (512 rows; one contiguous
   8KB run per partition) so every HWDGE load is 128 large descriptors --
   the SDMA per-descriptor fixed cost (~130ns/engine) makes smaller
   descriptors drain far below line rate.
 * head: weights then slab 0 on the Sync ring (which starts draining ~3us
   earlier than the Scalar ring); slab 1 and a single-copy bias ride the
   Scalar ring; the bias is broadcast on-chip.  PE warmup matmuls hold the
   HAM clock gate open until the data lands (~13.5 us; the framework
   preamble alone is ~6.6 us).
 * no bias matmul: the psum->SBUF eviction is a single Vector tensor_add
   per slab that adds the bias and narrows to fp16 (sole PSUM reader).
 * post-processing is batched per 4-subtile slab with h-major planes
   (mx/pe = [p, h, s, 150]) so exp (Scalar) and the product (Vector fp16
   2x) run on contiguous [p, 600] planes; segmented reduce over experts
   (Vector, no 2x mode exists); reciprocal per output GROUP (Vector) and
   the final num*rden on the otherwise idle GpSimd (contiguous fp32).
 * the LAST slab runs per-subtile (four 1-bank psum tiles via the pool's
   padded-shape rotation) so the tail chain after the final matmul is one
   subtile long, and rows are permuted (within each 2048-row group,
   partition p owns rows p*16..p*16+15) so each output store is 128 x 960B
   descriptors instead of 2048 x 60B.
"""

import sys

if "/opt/trn_rl_repo" not in sys.path:
    sys.path.insert(0, "/opt/trn_rl_repo")

import numpy as np

import concourse.bass as bass
import concourse.bacc as bacc
import concourse.tile as tile
import concourse.mybir as mybir
from concourse.bass_utils import run_bass_kernel_spmd

F32 = mybir.dt.float32
FP16 = mybir.dt.float16
BF16 = mybir.dt.bfloat16

MDT = FP16
NPDT = np.float16

BS = 65536
K = 1024
E = 10
O = 15
EO = E * O                # 150
NCOL = 2 * EO             # 300: cols 0..149 = expert (n=o*E+e), 150..299 = gate
NCORES = 8
RPC = BS // NCORES        # rows per core: 8192
KC = K // 128             # 8 contraction chunks
SLAB = 512                # rows per slab = 4 matmul subtiles
NSUB = SLAB // 128        # 4 subtiles per slab
NSLAB = RPC // SLAB       # 16 slabs per core
GROUP = 4                 # slabs per output group (2048 rows per out DMA)
NGRP = NSLAB // GROUP
PREFETCH = 5              # x slabs pre-issued before the main loop
N_WARMUP = 24             # PE warmup matmuls (HAM clock-gate release)
GP_FMUL = True            # final num*rden on GpSimd (False -> Vector)


def _build():
    nc = bacc.Bacc("TRN2", target_bir_lowering=False, debug=False,
                   num_devices=NCORES)
    # xt row k*128+q holds slab k's contiguous (j, c, m) block: j = subtile,
    # c = k-chunk, m = moving-row index p; q = k-chunk partition.
    xt = nc.dram_tensor("xt", [NSLAB * 128, NSUB * KC * 128], MDT,
                        kind="ExternalInput").ap()
    wt = nc.dram_tensor("wt", [128, KC * NCOL], MDT, kind="ExternalInput").ap()
    biasb = nc.dram_tensor("biasb", [128, NCOL], F32,
                           kind="ExternalInput").ap()
    out = nc.dram_tensor("out", [RPC, O], F32, kind="ExternalOutput").ap()

    with tile.TileContext(nc) as tc:
        with (
            tc.tile_pool(name="const", bufs=1) as cp,
            tc.tile_pool(name="x", bufs=PREFETCH + 2) as xp,
            tc.tile_pool(name="ps", bufs=2, space="PSUM") as ps_pool,
            tc.tile_pool(name="mx", bufs=2) as mx_pool,
            tc.tile_pool(name="pe", bufs=2) as pe_pool,
            tc.tile_pool(name="nd", bufs=2) as nd_pool,
            tc.tile_pool(name="ob", bufs=2) as ob_pool,
        ):
            # Weights then slab 0 on the Sync ring (earliest to drain); slab 1
            # plus the single-copy bias on the Scalar ring.
            wt_t = cp.tile([128, KC * NCOL], MDT, name="wt_t")
            nc.sync.dma_start(wt_t[:], wt[:])
            wt_v = wt_t[:].rearrange("p (c n) -> p c n", c=KC)
            xts = {}
            for k in range(min(PREFETCH + 1, NSLAB)):
                xt_t = xp.tile([128, NSUB * KC * 128], MDT, tag="xt",
                               name=f"xt_{k}")
                eng = nc.scalar if k == 1 else nc.sync
                eng.dma_start(xt_t[:], xt[k * 128:(k + 1) * 128, :])
                xts[k] = xt_t
            bias_sm = cp.tile([128, NCOL], F32, name="bias_sm")
            nc.scalar.dma_start(bias_sm[:], biasb[:])
            # Broadcast bias to all 4 subtile rows on the (idle) Scalar engine.
            biasb_t = cp.tile([128, NSUB, NCOL], F32, name="biasb_t")
            for j in range(NSUB):
                nc.scalar.copy(biasb_t[:, j, :], bias_sm[:])

            # Warm up the PE's HAM clock gate while the weights and slab 0
            # stream in: matmuls on a memset tile, no DMA deps.  The warmup
            # psum tile takes one rotation slot of the psum pool; its banks
            # are recycled later (same-engine WAW, no readers).
            wu_in = cp.tile([128, NCOL], BF16, name="wu_in")
            nc.gpsimd.memset(wu_in[:], 0.125)
            wu_ps = ps_pool.tile([128, NSUB * 512], F32, tag="ps", name="wu_ps")
            for _ in range(N_WARMUP):
                nc.tensor.matmul(
                    wu_ps[:, 0:NCOL], wu_in[:, 0:128], wu_in[:],
                    start=True, stop=True, skip_group_check=True,
                )

            ob = None
            ndb = None
            for k in range(NSLAB):
                kin = k % GROUP
                g = k // GROUP
                if kin == 0:
                    ob = ob_pool.tile([128, GROUP * NSUB * O], F32, tag="ob",
                                      name=f"ob_{g}")
                    # h-major: ndb[:, 0, :] = num plane, ndb[:, 1, :] = den.
                    ndb = nd_pool.tile([128, 2, GROUP * NSUB, O], F32,
                                       tag="ndb", name=f"ndb_{g}")
                kp = k + 1 + PREFETCH
                if kp < NSLAB:
                    xt_t = xp.tile([128, NSUB * KC * 128], MDT, tag="xt",
                                   name=f"xt_{kp}")
                    nc.sync.dma_start(xt_t[:], xt[kp * 128:(kp + 1) * 128, :])
                    xts[kp] = xt_t
                xt_v = xts[k][:].rearrange(
                    "p (j c m) -> p j c m", j=NSUB, c=KC)

                # h-major planes: [p, h, s, 150]; expert h=0, gate h=1.
                mx = mx_pool.tile([128, 2, NSUB, EO], MDT, tag="mx",
                                  name=f"mx_{k}")
                pe = pe_pool.tile([128, 2, NSUB, EO], MDT, tag="pe",
                                  name=f"pe_{k}")
                last = k == NSLAB - 1
                if not last:
                    psum = ps_pool.tile([128, NSUB * 512], F32, tag="ps",
                                        name=f"ps_{k}")
                    for j in range(NSUB):
                        for c in range(KC):
                            nc.tensor.matmul(
                                psum[:, j * 512:j * 512 + NCOL],
                                xt_v[:, j, c, :], wt_v[:, c, :],
                                start=(c == 0), stop=(c == KC - 1),
                            )
                    # Sole PSUM reader: evict + bias add + narrow to fp16.
                    # (iteration orders match: psum cols are h-major n, the
                    # mx view iterates (s, h, n); only free sizes must agree)
                    nc.vector.tensor_add(
                        mx[:].rearrange("p h s n -> p s h n"),
                        psum[:].rearrange("p (s b) -> p s b", s=NSUB)
                        [:, :, 0:NCOL],
                        biasb_t[:],
                    )
                    nc.scalar.activation(
                        pe[:, 1, :, :], mx[:, 1, :, :],
                        mybir.ActivationFunctionType.Exp,
                    )
                    nc.vector.tensor_mul(
                        pe[:, 0, :, :], mx[:, 0, :, :], pe[:, 1, :, :])
                    # Segmented sum over experts (e contiguous, n = o*E+e):
                    nc.vector.reduce_sum(
                        ndb[:, :, kin * NSUB:(kin + 1) * NSUB, :],
                        pe[:].rearrange("p h s (o e) -> p (h s) o e", o=O),
                        axis=mybir.AxisListType.X,
                    )
                else:
                    # Final slab: per-subtile pipeline (1-bank psum tiles via
                    # the pool's padded rotation) so the tail chain after the
                    # last matmul is one subtile long.
                    for j in range(NSUB):
                        psj = ps_pool.tile([128, 512], F32, tag="ps",
                                           name=f"ps_{k}_{j}",
                                           padded_shape=[128, NSUB * 512])
                        for c in range(KC):
                            nc.tensor.matmul(
                                psj[:, 0:NCOL],
                                xt_v[:, j, c, :], wt_v[:, c, :],
                                start=(c == 0), stop=(c == KC - 1),
                            )
                        nc.vector.tensor_add(
                            mx[:, :, j, :],
                            psj[:, 0:NCOL],
                            biasb_t[:, j, :],
                        )
                        nc.scalar.activation(
                            pe[:, 1, j, :], mx[:, 1, j, :],
                            mybir.ActivationFunctionType.Exp,
                        )
                        nc.vector.tensor_mul(
                            pe[:, 0, j, :], mx[:, 0, j, :], pe[:, 1, j, :])
                        nc.vector.reduce_sum(
                            ndb[:, :, kin * NSUB + j, :],
                            pe[:, :, j, :].rearrange("p h (o e) -> p h o e",
                                                     o=O),
                            axis=mybir.AxisListType.X,
                        )
                if kin == GROUP - 1:
                    rden = nd_pool.tile([128, GROUP * NSUB * O], F32,
                                        tag="rden", name=f"rden_{g}")
                    nc.vector.reciprocal_approx_fast(
                        rden[:], ndb[:, 1, :, :].rearrange("p s o -> p (s o)"))
                    feng = nc.gpsimd if GP_FMUL else nc.vector
                    feng.tensor_mul(
                        ob[:],
                        ndb[:, 0, :, :].rearrange("p s o -> p (s o)"),
                        rden[:])
                    g0 = g * GROUP * SLAB
                    # rows r = g0 + p*16 + s (host permutes x to match)
                    nc.scalar.dma_start(
                        out[g0:g0 + GROUP * SLAB, :]
                        .rearrange("(p s) o -> p (s o)", p=128),
                        ob[:],
                    )
    nc.compile()
    return nc


_NC = None


def _get_nc():
    global _NC
    if _NC is None:
        _NC = _build()
    return _NC


def _prep_inputs(x, expert_w, expert_b, gate_w, gate_b):
    # o-major expert columns (n = o*E + e) so the on-chip segmented reduce
    # over experts reads contiguous runs.
    w = np.concatenate([
        np.asarray(expert_w, np.float32).reshape(E, O, K)
        .transpose(1, 0, 2).reshape(EO, K),
        np.asarray(gate_w, np.float32).reshape(E, O, K)
        .transpose(1, 0, 2).reshape(EO, K),
    ], axis=0)                                   # [300, K], col n = o*E + e
    b = np.concatenate([
        np.asarray(expert_b, np.float32).reshape(E, O).T.reshape(EO),
        np.asarray(gate_b, np.float32).reshape(E, O).T.reshape(EO),
    ]).reshape(1, NCOL)
    # wt[q, (c, n)] = w[n, c*128+q]
    wt = np.ascontiguousarray(
        w.reshape(NCOL, KC, 128).transpose(2, 1, 0).astype(NPDT)
        .reshape(128, KC * NCOL))
    biasb = np.ascontiguousarray(
        np.broadcast_to(b, (128, NCOL)).astype(np.float32))
    # Row permutation: within each 2048-row group g of a core, partition p
    # owns rows g*2048 + p*16 + kin*4 + j (slab k = g*4+kin, subtile j).
    # Moving-row index m = p; block layout per slab-row q is (j, c, m).
    x16 = np.asarray(x).astype(NPDT)
    arr = x16.reshape(NCORES, NGRP, 128, GROUP, NSUB, KC, 128)
    #                 core    g     p    kin    j    c   q
    xt = np.ascontiguousarray(arr.transpose(0, 1, 3, 6, 4, 5, 2)) \
        .reshape(NCORES, NSLAB * 128, NSUB * KC * 128)
    in_maps = [{"xt": xt[i], "wt": wt, "biasb": biasb}
               for i in range(NCORES)]
    return in_maps


def _run(in_maps, **kw):
    res = run_bass_kernel_spmd(
        _get_nc(), in_maps, core_ids=list(range(NCORES)), **kw)
    out = np.concatenate([r["out"] for r in res.results], axis=0)
    return out, res


def kernel(x, expert_w, expert_b, gate_w, gate_b):
    in_maps = _prep_inputs(x, expert_w, expert_b, gate_w, gate_b)
    out, _ = _run(in_maps)
    return out


def kernel_traced(x, expert_w, expert_b, gate_w, gate_b, **kw):
    """Like kernel() but returns (out, BassKernelResults) with an NTFF trace."""
    in_maps = _prep_inputs(x, expert_w, expert_b, gate_w, gate_b)
    return _run(in_maps, trace=True, **kw)


# revision 12
# speedup vs baseline: 1.0256x; 1.0256x over previous
"""Trainium2 Bass kernel for nn_MixtureOfExpertsModel (moe_routing).

Computes, for x [65536, 1024] and 10 experts with 15 outputs each:
    miu = x @ expert_w^T + expert_b      (per expert)
    xi  = x @ gate_w^T + gate_b          (per expert)
    out = sum_e softmax_e(xi) * miu      -> [65536, 15]

Strategy: pure data parallel over 8 NeuronCores (8192 rows each); at fp16
the kernel is PE-bound (2.52 GMAC/core -> 153.6k PE cycles = 64 us), so the
design keeps the PE streaming 300-column fp16 matmuls back-to-back at the
~127 ns issue floor and pushes everything else off the critical path:

 * x is repacked on the host into per-slab blocks (512 rows; one contiguous
   8KB run per partition) so every HWDGE load is 128 large descriptors --
   the SDMA per-descriptor fixed cost (~130ns/engine) makes smaller
   descriptors drain far below line rate.
 * head: the first real matmul needs the weights and slab 0, which drain at
   ~8.1us (Sync ring) / ~9.9us (Scalar ring) earliest.  Both are split in
   half across the two rings (wtA+slab0a+slab1 on Sync; wtB+bias+slab0b+
   slab2 on Scalar) so group 0 can start at ~14us; 40 PE warmup matmuls
   hold the HAM clock gate open until then (a >3.4us idle gap would drop
   the PE back to 1.2GHz for ~35 real matmuls).
 * no bias matmul: the psum->SBUF eviction is a single Vector tensor_add
   per slab that adds the bias and narrows to fp16 (sole PSUM reader).
 * post-processing is batched per 4-subtile slab with h-major planes
   (mx/pe = [p, h, s, 150]) so exp (Scalar) and the product (Vector fp16
   2x) run on contiguous [p, 600] planes; segmented reduce over experts
   (Vector; tensor_reduce has no DVE 2x mode); reciprocal per output GROUP
   (Vector) and the final num*rden on the otherwise idle GpSimd for groups
   0-2; the last group runs its finals per slab on Vector.
 * the last TWO slabs run per-subtile (1-bank psum tiles via the pool's
   padded rotation) so PSUM hand-back is smooth at the end and the tail
   chain after the final matmul is one subtile long; rows are permuted
   (within each 2048-row group, partition p owns rows p*16..p*16+15) so
   each output store is 128 x 960B descriptors instead of 2048 x 60B.
"""

import sys

if "/opt/trn_rl_repo" not in sys.path:
    sys.path.insert(0, "/opt/trn_rl_repo")

import numpy as np

import concourse.bass as bass
import concourse.bacc as bacc
import concourse.tile as tile
import concourse.mybir as mybir
from concourse.bass_utils import run_bass_kernel_spmd

F32 = mybir.dt.float32
FP16 = mybir.dt.float16
BF16 = mybir.dt.bfloat16

MDT = FP16
NPDT = np.float16

BS = 65536
K = 1024
E = 10
O = 15
EO = E * O                # 150
NCOL = 2 * EO             # 300: cols 0..149 = expert (n=o*E+e), 150..299 = gate
NCORES = 8
RPC = BS // NCORES        # rows per core: 8192
KC = K // 128             # 8 contraction chunks
SLAB = 512                # rows per slab = 4 matmul subtiles
NSUB = SLAB // 128        # 4 subtiles per slab
NSLAB = RPC // SLAB       # 16 slabs per core
GROUP = 4                 # slabs per output group (2048 rows per out DMA)
NGRP = NSLAB // GROUP
PREFETCH = 5              # x slabs in flight ahead of compute
N_WARMUP = 40             # PE warmup matmuls (HAM clock-gate release)
FINE = (NSLAB - 2, NSLAB - 1)   # slabs run per-subtile (tail smoothing)


def _build():
    nc = bacc.Bacc("TRN2", target_bir_lowering=False, debug=False,
                   num_devices=NCORES)
    # xt row k*128+q holds slab k's contiguous (j, c, m) block: j = subtile,
    # c = k-chunk, m = moving-row index p; q = k-chunk partition.
    xt = nc.dram_tensor("xt", [NSLAB * 128, NSUB * KC * 128], MDT,
                        kind="ExternalInput").ap()
    wt = nc.dram_tensor("wt", [128, KC * NCOL], MDT, kind="ExternalInput").ap()
    biasb = nc.dram_tensor("biasb", [128, NCOL], MDT,
                           kind="ExternalInput").ap()
    out = nc.dram_tensor("out", [RPC, O], F32, kind="ExternalOutput").ap()

    HKC = KC // 2
    HX = 2 * KC * 128     # half-slab elements per partition

    with tile.TileContext(nc) as tc:
        with (
            tc.tile_pool(name="const", bufs=1) as cp,
            tc.tile_pool(name="x0", bufs=1) as x0p,
            tc.tile_pool(name="x", bufs=PREFETCH + 2) as xp,
            tc.tile_pool(name="ps", bufs=2, space="PSUM") as ps_pool,
            tc.tile_pool(name="mx", bufs=2) as mx_pool,
            tc.tile_pool(name="pe", bufs=2) as pe_pool,
            tc.tile_pool(name="nd", bufs=2) as nd_pool,
            tc.tile_pool(name="ob", bufs=2) as ob_pool,
        ):
            # Sync ring: wtA (k-chunks 0-3), slab0 subtiles 0-1, slab 1.
            # Scalar ring: wtB, bias (fp16, tiny), slab0 subtiles 2-3, slab 2.
            wt_t = cp.tile([128, KC * NCOL], MDT, name="wt_t")
            nc.sync.dma_start(wt_t[:, 0:HKC * NCOL], wt[:, 0:HKC * NCOL])
            nc.scalar.dma_start(wt_t[:, HKC * NCOL:], wt[:, HKC * NCOL:])
            wt_v = wt_t[:].rearrange("p (c n) -> p c n", c=KC)

            s0a = x0p.tile([128, HX], MDT, name="s0a")
            nc.sync.dma_start(s0a[:], xt[0:128, 0:HX])
            bias_sm = cp.tile([128, NCOL], MDT, name="bias_sm")
            nc.scalar.dma_start(bias_sm[:], biasb[:])
            s0b = x0p.tile([128, HX], MDT, name="s0b")
            nc.scalar.dma_start(s0b[:], xt[0:128, HX:2 * HX])
            s0v = [s0a[:].rearrange("p (j c m) -> p j c m", j=2, c=KC),
                   s0b[:].rearrange("p (j c m) -> p j c m", j=2, c=KC)]

            xts = {}
            for k in (1, 2):
                xt_t = xp.tile([128, NSUB * KC * 128], MDT, tag="xt",
                               name=f"xt_{k}")
                eng = nc.sync if k == 1 else nc.scalar
                eng.dma_start(xt_t[:], xt[k * 128:(k + 1) * 128, :])
                xts[k] = xt_t
            for k in range(3, min(1 + PREFETCH, NSLAB)):
                xt_t = xp.tile([128, NSUB * KC * 128], MDT, tag="xt",
                               name=f"xt_{k}")
                nc.sync.dma_start(xt_t[:], xt[k * 128:(k + 1) * 128, :])
                xts[k] = xt_t

            # Broadcast bias to all 4 subtile rows (fp16 -> fp32) on the
            # (idle at head) Scalar engine.
            biasb_t = cp.tile([128, NSUB, NCOL], F32, name="biasb_t")
            for j in range(NSUB):
                nc.scalar.copy(biasb_t[:, j, :], bias_sm[:])

            # Warm up the PE's HAM clock gate while the weights and slab 0
            # stream in: matmuls on a memset tile, no DMA deps.
            wu_in = cp.tile([128, NCOL], BF16, name="wu_in")
            nc.gpsimd.memset(wu_in[:], 0.125)
            wu_ps = ps_pool.tile([128, NSUB * 512], F32, tag="ps", name="wu_ps")
            for _ in range(N_WARMUP):
                nc.tensor.matmul(
                    wu_ps[:, 0:NCOL], wu_in[:, 0:128], wu_in[:],
                    start=True, stop=True, skip_group_check=True,
                )

            ob = None
            ndb = None
            for k in range(NSLAB):
                kin = k % GROUP
                g = k // GROUP
                if kin == 0:
                    ob = ob_pool.tile([128, GROUP * NSUB * O], F32, tag="ob",
                                      name=f"ob_{g}")
                    # h-major: ndb[:, 0, :] = num plane, ndb[:, 1, :] = den.
                    ndb = nd_pool.tile([128, 2, GROUP * NSUB, O], F32,
                                       tag="ndb", name=f"ndb_{g}")
                kp = k + 1 + PREFETCH
                if kp < NSLAB:
                    xt_t = xp.tile([128, NSUB * KC * 128], MDT, tag="xt",
                                   name=f"xt_{kp}")
                    nc.sync.dma_start(xt_t[:], xt[kp * 128:(kp + 1) * 128, :])
                    xts[kp] = xt_t

                def stat(j, c):
                    if k == 0:
                        return s0v[j // 2][:, j % 2, c, :]
                    return xts[k][:].rearrange(
                        "p (j c m) -> p j c m", j=NSUB, c=KC)[:, j, c, :]

                # h-major planes: [p, h, s, 150]; expert h=0, gate h=1.
                mx = mx_pool.tile([128, 2, NSUB, EO], MDT, tag="mx",
                                  name=f"mx_{k}")
                pe = pe_pool.tile([128, 2, NSUB, EO], MDT, tag="pe",
                                  name=f"pe_{k}")
                if k not in FINE:
                    psum = ps_pool.tile([128, NSUB * 512], F32, tag="ps",
                                        name=f"ps_{k}")
                    for j in range(NSUB):
                        for c in range(KC):
                            nc.tensor.matmul(
                                psum[:, j * 512:j * 512 + NCOL],
                                stat(j, c), wt_v[:, c, :],
                                start=(c == 0), stop=(c == KC - 1),
                            )
                    # Sole PSUM reader: evict + bias add + narrow to fp16.
                    # (iteration orders match: psum cols are h-major n, the
                    # mx view iterates (s, h, n); only free sizes must agree)
                    nc.vector.tensor_add(
                        mx[:].rearrange("p h s n -> p s h n"),
                        psum[:].rearrange("p (s b) -> p s b", s=NSUB)
                        [:, :, 0:NCOL],
                        biasb_t[:],
                    )
                    nc.scalar.activation(
                        pe[:, 1, :, :], mx[:, 1, :, :],
                        mybir.ActivationFunctionType.Exp,
                    )
                    nc.vector.tensor_mul(
                        pe[:, 0, :, :], mx[:, 0, :, :], pe[:, 1, :, :])
                    # Segmented sum over experts (e contiguous, n = o*E+e):
                    nc.vector.reduce_sum(
                        ndb[:, :, kin * NSUB:(kin + 1) * NSUB, :],
                        pe[:].rearrange("p h s (o e) -> p (h s) o e", o=O),
                        axis=mybir.AxisListType.X,
                    )
                else:
                    # Tail slabs: per-subtile pipeline (1-bank psum tiles via
                    # the pool's padded rotation) so PSUM hand-back stays
                    # smooth and the post-matmul chain is one subtile long.
                    for j in range(NSUB):
                        psj = ps_pool.tile([128, 512], F32, tag="ps",
                                           name=f"ps_{k}_{j}",
                                           padded_shape=[128, NSUB * 512])
                        for c in range(KC):
                            nc.tensor.matmul(
                                psj[:, 0:NCOL],
                                stat(j, c), wt_v[:, c, :],
                                start=(c == 0), stop=(c == KC - 1),
                            )
                        nc.vector.tensor_add(
                            mx[:, :, j, :], psj[:, 0:NCOL], biasb_t[:, j, :])
                        nc.scalar.activation(
                            pe[:, 1, j, :], mx[:, 1, j, :],
                            mybir.ActivationFunctionType.Exp,
                        )
                        nc.vector.tensor_mul(
                            pe[:, 0, j, :], mx[:, 0, j, :], pe[:, 1, j, :])
                        nc.vector.reduce_sum(
                            ndb[:, :, kin * NSUB + j, :],
                            pe[:, :, j, :].rearrange("p h (o e) -> p h o e",
                                                     o=O),
                            axis=mybir.AxisListType.X,
                        )
                if g < NGRP - 1:
                    if kin == GROUP - 1:
                        rden = nd_pool.tile([128, GROUP * NSUB * O], F32,
                                            tag="rden", name=f"rden_{g}")
                        nc.vector.reciprocal_approx_fast(
                            rden[:],
                            ndb[:, 1, :, :].rearrange("p s o -> p (s o)"))
                        nc.gpsimd.tensor_mul(
                            ob[:],
                            ndb[:, 0, :, :].rearrange("p s o -> p (s o)"),
                            rden[:])
                else:
                    # Last group: per-slab finals on Vector so the tail chain
                    # after the last reduce is short.
                    if kin == 0:
                        rden = nd_pool.tile([128, GROUP * NSUB * O], F32,
                                            tag="rden", name=f"rden_{g}")
                    sl = slice(kin * NSUB * O, (kin + 1) * NSUB * O)
                    nc.vector.reciprocal_approx_fast(
                        rden[:, sl],
                        ndb[:, 1, kin * NSUB:(kin + 1) * NSUB, :]
                        .rearrange("p s o -> p (s o)"))
                    nc.vector.tensor_mul(
                        ob[:, sl],
                        ndb[:, 0, kin * NSUB:(kin + 1) * NSUB, :]
                        .rearrange("p s o -> p (s o)"),
                        rden[:, sl])
                if kin == GROUP - 1:
                    g0 = g * GROUP * SLAB
                    # rows r = g0 + p*16 + s (host permutes x to match)
                    nc.scalar.dma_start(
                        out[g0:g0 + GROUP * SLAB, :]
                        .rearrange("(p s) o -> p (s o)", p=128),
                        ob[:],
                    )
    nc.compile()
    return nc


_NC = None


def _get_nc():
    global _NC
    if _NC is None:
        _NC = _build()
    return _NC


def _prep_inputs(x, expert_w, expert_b, gate_w, gate_b):
    # o-major expert columns (n = o*E + e) so the on-chip segmented reduce
    # over experts reads contiguous runs.
    w = np.concatenate([
        np.asarray(expert_w, np.float32).reshape(E, O, K)
        .transpose(1, 0, 2).reshape(EO, K),
        np.asarray(gate_w, np.float32).reshape(E, O, K)
        .transpose(1, 0, 2).reshape(EO, K),
    ], axis=0)                                   # [300, K], col n = o*E + e
    b = np.concatenate([
        np.asarray(expert_b, np.float32).reshape(E, O).T.reshape(EO),
        np.asarray(gate_b, np.float32).reshape(E, O).T.reshape(EO),
    ]).reshape(1, NCOL)
    # wt[q, (c, n)] = w[n, c*128+q]
    wt = np.ascontiguousarray(
        w.reshape(NCOL, KC, 128).transpose(2, 1, 0).astype(NPDT)
        .reshape(128, KC * NCOL))
    biasb = np.ascontiguousarray(
        np.broadcast_to(b, (128, NCOL)).astype(NPDT))
    # Row permutation: within each 2048-row group g of a core, partition p
    # owns rows g*2048 + p*16 + kin*4 + j (slab k = g*4+kin, subtile j).
    # Moving-row index m = p; block layout per slab-row q is (j, c, m).
    x16 = np.asarray(x).astype(NPDT)
    arr = x16.reshape(NCORES, NGRP, 128, GROUP, NSUB, KC, 128)
    #                 core    g     p    kin    j    c   q
    xt = np.ascontiguousarray(arr.transpose(0, 1, 3, 6, 4, 5, 2)) \
        .reshape(NCORES, NSLAB * 128, NSUB * KC * 128)
    in_maps = [{"xt": xt[i], "wt": wt, "biasb": biasb}
               for i in range(NCORES)]
    return in_maps


def _run(in_maps, **kw):
    res = run_bass_kernel_spmd(
        _get_nc(), in_maps, core_ids=list(range(NCORES)), **kw)
    out = np.concatenate([r["out"] for r in res.results], axis=0)
    return out, res


def kernel(x, expert_w, expert_b, gate_w, gate_b):
    in_maps = _prep_inputs(x, expert_w, expert_b, gate_w, gate_b)
    out, _ = _run(in_maps)
    return out


def kernel_traced(x, expert_w, expert_b, gate_w, gate_b, **kw):
    """Like kernel() but returns (out, BassKernelResults) with an NTFF trace."""
    in_maps = _prep_inputs(x, expert_w, expert_b, gate_w, gate_b)
    return _run(in_maps, trace=True, **kw)
